# revision 30
# baseline (speedup 1.0000x reference)
"""Trainium2 Bass kernel for nn_DecoderLayer (GNN message passing decoder layer).

Math (per reference):
  seq_j = seq_emb[edge_idx] * ar_mask[..., None]
  x = concat([h_i, h_j, edge_h, seq_j], -1)            # [res,k,4h]
  msg = gelu(x @ mW1 + mb1); msg = gelu(msg @ mW2 + mb2); msg = msg @ mW3 + mb3
  agg = msg.sum(1)
  h = LN(node_h + agg) * g1 + b1
  ff = gelu(h @ fW1 + fb1) @ fW2 + fb2
  h = LN(h + ff) * g2 + b2

Strategy (8-way data parallel over the residue dim, no collectives):
  - mm1 is decomposed: x@mW1 = h_i@Wa + h_j@Wb + edge_h@Wc + seq_j@Wd.
    h_j@Wb and seq_emb@Wd are precomputed per *global* node into a fused
    fp8e4m3 gather table [8192, 256]; per-edge rows are fetched with
    dma_gather (alternating SWDGE queues so Q7 descriptor prep overlaps
    the DMA drain) and transpose-accumulated into PSUM via identity
    matmuls; the ar_mask folds into the seq-half transpose as a
    diag(mask) rhs built on the vector engine.
  - k-reduction is moved before mm3 (linearity): 48x less mm3 work.
  - activations live feature-major ("T layout", [feat, rows]); edge_h,
    node_hT and seqT are host-cast to bf16 to halve HBM traffic, and the
    big resident loads are split across engine queues for bandwidth.
  - phase 3 (mm3/LN1/FF/LN2) is interleaved into the edge-chunk loop so
    it overlaps with later chunks' gathers.
"""

import sys

sys.path.insert(0, "/opt/trn_rl_repo")

import numpy as np
import ml_dtypes

import concourse.bacc as bacc
import concourse.bass as bass
import concourse.mybir as mybir
import concourse.tile as tile
from concourse import bass_utils

BF16 = ml_dtypes.bfloat16
F32 = mybir.dt.float32
BF = mybir.dt.bfloat16
I16 = mybir.dt.int16
F8 = mybir.dt.float8e4

RES, KK, H = 8192, 48, 128
N_CORES = 8
RT = 384  # rows per psum tile (8 nodes x 48 edges)


def build_nc(n_glob, n_loc, num_devices, chunk_tiles=2):
    """Build the bass program for one core holding n_loc nodes of an
    n_glob-node graph. All sizes in nodes; n_loc % 128 == 0, n_glob % 128 == 0."""
    E = n_loc * KK
    assert E % RT == 0
    T = E // RT  # number of 384-row tiles
    n_ch = (T + chunk_tiles - 1) // chunk_tiles
    assert T % n_ch == 0
    cht = T // n_ch  # tiles per chunk
    ch_rows = cht * RT  # rows per chunk (gather num_idxs)
    nblk = n_loc // 128  # node blocks
    gblk = n_glob // 128  # global node blocks (table build)
    ch_per_blk = n_ch // nblk  # chunks per 128-node block

    nc = bacc.Bacc("TRN2", target_bir_lowering=False, debug=False,
                   num_devices=num_devices, num_swdge_queues=4)

    def din(name, shape, dt):
        return nc.dram_tensor(name, shape, dt, kind="ExternalInput")

    edge_hT = din("edge_hT", [H, E], BF)
    idx16 = din("idx16", [128, E // 16], I16)
    maskc = din("maskc", [128, 3 * T], F32)
    node_hT = din("node_hT", [H, n_glob], BF)
    seqT = din("seqT", [H, n_glob], BF)
    nhl = din("nhl", [128, nblk, H], F32)
    wa = din("wa", [H, H], BF)
    wb = din("wb", [H, H], BF)
    wc = din("wc", [H, H], BF)
    wd = din("wd", [H, H], BF)
    w2 = din("w2", [H, H], BF)
    w3 = din("w3", [H, H], BF)
    fw1 = din("fw1", [H, 4 * H], BF)
    fw2 = din("fw2", [H, 4, H], BF)
    ident = din("ident", [128, 128], BF)
    mb1c = din("mb1c", [H, 1], F32)
    mb2c = din("mb2c", [H, 1], F32)
    mb3x48 = din("mb3x48", [H, 1], F32)
    fb1c = din("fb1c", [H, 4], F32)
    fb2c = din("fb2c", [H, 1], F32)
    g1bc = din("g1bc", [128, H], BF)
    b1bc = din("b1bc", [128, H], BF)
    g2bc = din("g2bc", [128, H], F32)
    b2bc = din("b2bc", [128, H], F32)
    out = nc.dram_tensor("out", [n_loc, H], F32, kind="ExternalOutput")

    GELU = mybir.ActivationFunctionType.Gelu
    IDENT = mybir.ActivationFunctionType.Identity
    COPY = mybir.ActivationFunctionType.Copy
    SQRT = mybir.ActivationFunctionType.Sqrt
    AX = mybir.AxisListType.X
    SUB = mybir.AluOpType.subtract
    MUL = mybir.AluOpType.mult

    with tile.TileContext(nc) as tc:
        with tc.tile_pool(name="singles", bufs=1) as sg, \
             tc.tile_pool(name="dram", bufs=1, space="DRAM") as dp:
            # ---- resident tiles; big loads are split across engine queues ----
            qs = [nc.sync, nc.scalar]
            s_idx = sg.tile([128, E // 16], I16)
            nq = E // 16 // 4
            for i in range(4):
                qs[i % 2].dma_start(out=s_idx[:, nq * i:nq * (i + 1)],
                                    in_=idx16.ap()[:, nq * i:nq * (i + 1)])
            nhT_bf = sg.tile([128, n_glob], BF)
            seT_bf = sg.tile([128, n_glob], BF)
            gq = n_glob // 16
            for i in range(16):
                qs[i % 2].dma_start(out=nhT_bf[:, gq * i:gq * (i + 1)],
                                    in_=node_hT.ap()[:, gq * i:gq * (i + 1)])
                qs[(i + 1) % 2].dma_start(out=seT_bf[:, gq * i:gq * (i + 1)],
                                          in_=seqT.ap()[:, gq * i:gq * (i + 1)])
            s_wb = sg.tile([H, H], BF)
            nc.sync.dma_start(out=s_wb[:], in_=wb.ap())
            s_wd = sg.tile([H, H], BF)
            nc.sync.dma_start(out=s_wd[:], in_=wd.ap())
            s_id = sg.tile([128, 128], BF)
            nc.sync.dma_start(out=s_id[:], in_=ident.ap())

            def late_loads():
                d = {}
                d["s_maskc"] = sg.tile([128, 3 * T], F32, name="s_maskc")
                nc.sync.dma_start(out=d["s_maskc"][:], in_=maskc.ap())
                d["s_nhl"] = sg.tile([128, nblk, H], F32, name="s_nhl")
                nc.scalar.dma_start(out=d["s_nhl"][:], in_=nhl.ap())
                d["s_wa"] = sg.tile([H, H], BF, name="s_wa")
                nc.sync.dma_start(out=d["s_wa"][:], in_=wa.ap())
                d["s_wc"] = sg.tile([H, H], BF, name="s_wc")
                nc.sync.dma_start(out=d["s_wc"][:], in_=wc.ap())
                d["s_w2"] = sg.tile([H, H], BF, name="s_w2")
                nc.sync.dma_start(out=d["s_w2"][:], in_=w2.ap())
                d["s_w3"] = sg.tile([H, H], BF, name="s_w3")
                nc.sync.dma_start(out=d["s_w3"][:], in_=w3.ap())
                d["s_fw1"] = sg.tile([H, 4 * H], BF, name="s_fw1")
                nc.scalar.dma_start(out=d["s_fw1"][:], in_=fw1.ap())
                d["s_fw2"] = sg.tile([H, 4, H], BF, name="s_fw2")
                nc.scalar.dma_start(out=d["s_fw2"][:], in_=fw2.ap())
                d["s_mb1c"] = sg.tile([H, 1], F32, name="s_mb1c")
                nc.sync.dma_start(out=d["s_mb1c"][:], in_=mb1c.ap())
                d["s_mb2c"] = sg.tile([H, 1], F32, name="s_mb2c")
                nc.sync.dma_start(out=d["s_mb2c"][:], in_=mb2c.ap())
                d["s_mb3x48"] = sg.tile([H, 1], F32, name="s_mb3x48")
                nc.sync.dma_start(out=d["s_mb3x48"][:], in_=mb3x48.ap())
                d["s_fb1c"] = sg.tile([H, 4], F32, name="s_fb1c")
                nc.sync.dma_start(out=d["s_fb1c"][:], in_=fb1c.ap())
                d["s_fb2c"] = sg.tile([H, 1], F32, name="s_fb2c")
                nc.sync.dma_start(out=d["s_fb2c"][:], in_=fb2c.ap())
                d["s_g1bc"] = sg.tile([128, H], BF, name="s_g1bc")
                nc.scalar.dma_start(out=d["s_g1bc"][:], in_=g1bc.ap())
                d["s_b1bc"] = sg.tile([128, H], BF, name="s_b1bc")
                nc.scalar.dma_start(out=d["s_b1bc"][:], in_=b1bc.ap())
                d["s_g2bc"] = sg.tile([128, H], F32, name="s_g2bc")
                nc.scalar.dma_start(out=d["s_g2bc"][:], in_=g2bc.ap())
                d["s_b2bc"] = sg.tile([128, H], F32, name="s_b2bc")
                nc.scalar.dma_start(out=d["s_b2bc"][:], in_=b2bc.ap())
                d["s_eps"] = sg.tile([128, 1], F32, name="s_eps")
                nc.vector.memset(d["s_eps"][:], 1e-5)
                d["s_nhTl"] = sg.tile([128, n_loc], BF, name="s_nhTl")
                nc.vector.tensor_copy(out=d["s_nhTl"][:], in_=nhT_bf[:, 0:n_loc])
                return d
            s_aggT = sg.tile([128, n_loc], F32)
            s_aggTb = sg.tile([128, n_loc], BF)
            s_a2Tb = sg.tile([128, n_loc], BF)
            s_h1T = sg.tile([128, n_loc], BF)
            s_h1rm = sg.tile([128, nblk, H], BF)

            table = dp.tile([n_glob, 256], F8)

            # ---- phase 1: gather table build (deep pipeline) ----
            with tc.tile_pool(name="p1s", bufs=6) as p1s, \
                 tc.tile_pool(name="p1p", bufs=6, space="PSUM") as p1p:
                for b in range(gblk):
                    ps = p1p.tile([128, 256], F32, tag="tps")
                    nc.tensor.matmul(out=ps[:, 0:128],
                                     lhsT=nhT_bf[:, 128 * b:128 * (b + 1)],
                                     rhs=s_wb[:], start=True, stop=True)
                    nc.tensor.matmul(out=ps[:, 128:256],
                                     lhsT=seT_bf[:, 128 * b:128 * (b + 1)],
                                     rhs=s_wd[:], start=True, stop=True)
                    tb = p1s.tile([128, 256], F8, tag="tb")
                    if b % 2 == 0:
                        nc.vector.tensor_copy(out=tb[:], in_=ps[:])
                    else:
                        nc.scalar.activation(out=tb[:], in_=ps[:], func=COPY)
                    (qs + [nc.gpsimd])[b % 3].dma_start(
                        out=table[128 * b:128 * (b + 1), :], in_=tb[:])

            lt = late_loads()
            s_maskc = lt["s_maskc"]; s_nhl = lt["s_nhl"]
            s_wa = lt["s_wa"]; s_wc = lt["s_wc"]; s_w2 = lt["s_w2"]
            s_w3 = lt["s_w3"]; s_fw1 = lt["s_fw1"]; s_fw2 = lt["s_fw2"]
            s_mb1c = lt["s_mb1c"]; s_mb2c = lt["s_mb2c"]
            s_mb3x48 = lt["s_mb3x48"]; s_fb1c = lt["s_fb1c"]
            s_fb2c = lt["s_fb2c"]; s_g1bc = lt["s_g1bc"]
            s_b1bc = lt["s_b1bc"]; s_g2bc = lt["s_g2bc"]
            s_b2bc = lt["s_b2bc"]; s_eps = lt["s_eps"]
            s_nhTl = lt["s_nhTl"]

            # ---- phase 2 + interleaved phase 3 ----
            with tc.tile_pool(name="p2g", bufs=5) as p2g, \
                 tc.tile_pool(name="p2e", bufs=4) as p2e, \
                 tc.tile_pool(name="p2s", bufs=3) as p2s, \
                 tc.tile_pool(name="pp1", bufs=2, space="PSUM") as pp1, \
                 tc.tile_pool(name="pp2", bufs=2, space="PSUM") as pp2, \
                 tc.tile_pool(name="p3s", bufs=2) as p3s, \
                 tc.tile_pool(name="p3o", bufs=2) as p3o, \
                 tc.tile_pool(name="pp3", bufs=2, space="PSUM") as pp3, \
                 tc.tile_pool(name="pp4", bufs=1, space="PSUM") as pp4:

                def newton_rstd(mv, tagp):
                    """rstd = (var+eps)^-1/2 on DVE only (no ACT table swap):
                    reciprocal seed + 5 Newton iterations."""
                    vh = p3s.tile([128, 1], F32, tag=tagp + "vh")
                    nc.vector.tensor_scalar(out=vh[:], in0=mv[:, 1:2],
                                            scalar1=0.5, scalar2=5e-6,
                                            op0=MUL, op1=mybir.AluOpType.add)
                    y = p3s.tile([128, 1], F32, tag=tagp + "y")
                    nc.vector.reciprocal(out=y[:], in_=mv[:, 1:2])
                    for _ in range(5):
                        t = p3s.tile([128, 1], F32, tag=tagp + "t")
                        nc.vector.tensor_mul(out=t[:], in0=y[:], in1=y[:])
                        nc.vector.tensor_mul(out=t[:], in0=t[:], in1=vh[:])
                        nc.vector.tensor_scalar(out=t[:], in0=t[:],
                                                scalar1=-1.0, scalar2=1.5,
                                                op0=MUL,
                                                op1=mybir.AluOpType.add)
                        nc.vector.tensor_mul(out=y[:], in0=y[:], in1=t[:])
                    return y

                def phase3_blockA(b):
                    """mm3 + LN1 for 128-node block b."""
                    psm = pp3.tile([128, 128], F32, tag="p3t")
                    nc.tensor.matmul(out=psm[:], lhsT=s_w3[:],
                                     rhs=s_aggTb[:, 128 * b:128 * (b + 1)],
                                     start=True, stop=True)
                    nc.scalar.activation(out=s_a2Tb[:, 128 * b:128 * (b + 1)],
                                         in_=psm[:], func=IDENT,
                                         bias=s_mb3x48[:])
                    # LN1
                    psrm = pp3.tile([128, 128], F32, tag="p3t")
                    nc.tensor.matmul(out=psrm[:],
                                     lhsT=s_a2Tb[:, 128 * b:128 * (b + 1)],
                                     rhs=s_id[:], start=True, stop=True)
                    x1 = p3s.tile([128, 128], F32, tag="x1")
                    nc.vector.tensor_add(out=x1[:], in0=psrm[:],
                                         in1=s_nhl[:, b, :])
                    st = p3s.tile([128, 6], F32, tag="st")
                    nc.vector.bn_stats(out=st[:], in_=x1[:])
                    mv = p3s.tile([128, 2], F32, tag="mv")
                    nc.vector.bn_aggr(out=mv[:], in_=st[:])
                    sd = p3s.tile([128, 1], F32, tag="sd")
                    nc.scalar.activation(out=sd[:], in_=mv[:, 1:2], func=SQRT,
                                         bias=s_eps[:])
                    rstd = p3s.tile([128, 1], F32, tag="rstd")
                    nc.vector.reciprocal(out=rstd[:], in_=sd[:])
                    xn = p3s.tile([128, 128], BF, tag="xn")
                    nc.vector.tensor_scalar(out=xn[:], in0=x1[:],
                                            scalar1=mv[:, 0:1], scalar2=rstd[:],
                                            op0=SUB, op1=MUL)
                    tb1 = p3s.tile([128, 128], BF, tag="tb1")
                    nc.vector.tensor_mul(out=tb1[:], in0=xn[:], in1=s_g1bc[:])
                    nc.vector.tensor_add(out=s_h1rm[:, b, :], in0=tb1[:],
                                         in1=s_b1bc[:])

                def phase3_blockA2(b):
                    """h1 transpose for block b (inputs one chunk old)."""
                    psT = pp3.tile([128, 128], F32, tag="p3t")
                    nc.tensor.matmul(out=psT[:], lhsT=s_h1rm[:, b, :],
                                     rhs=s_id[:], start=True, stop=True)
                    nc.scalar.activation(out=s_h1T[:, 128 * b:128 * (b + 1)],
                                         in_=psT[:], func=COPY)

                def phase3_blockB(b):
                    """FF + LN2 + out DMA for 128-node block b."""
                    us = []
                    for fc in range(4):
                        psf = pp3.tile([128, 128], F32, tag="p3t")
                        nc.tensor.matmul(out=psf[:],
                                         lhsT=s_fw1[:, 128 * fc:128 * (fc + 1)],
                                         rhs=s_h1T[:, 128 * b:128 * (b + 1)],
                                         start=True, stop=True)
                        u = p3s.tile([128, 128], BF, tag=f"u{fc}")
                        nc.scalar.activation(out=u[:], in_=psf[:], func=GELU,
                                             bias=s_fb1c[:, fc:fc + 1])
                        us.append(u)
                    psf2 = pp4.tile([128, 128], F32, tag="psf2")
                    for fc in range(4):
                        nc.tensor.matmul(out=psf2[:], lhsT=s_fw2[:, fc, :],
                                         rhs=us[fc][:], start=(fc == 0),
                                         stop=(fc == 3))
                    ffT = p3s.tile([128, 128], BF, tag="ffT")
                    nc.scalar.activation(out=ffT[:], in_=psf2[:],
                                         func=IDENT, bias=s_fb2c[:])
                    psr2 = pp3.tile([128, 128], F32, tag="p3t")
                    nc.tensor.matmul(out=psr2[:], lhsT=ffT[:], rhs=s_id[:],
                                     start=True, stop=True)
                    ffrm = p3s.tile([128, 128], BF, tag="ffrm")
                    nc.scalar.activation(out=ffrm[:], in_=psr2[:], func=COPY)
                    x2 = p3s.tile([128, 128], F32, tag="x2")
                    nc.vector.tensor_add(out=x2[:], in0=ffrm[:],
                                         in1=s_h1rm[:, b, :])
                    st2 = p3s.tile([128, 6], F32, tag="st2")
                    nc.vector.bn_stats(out=st2[:], in_=x2[:])
                    mv2 = p3s.tile([128, 2], F32, tag="mv2")
                    nc.vector.bn_aggr(out=mv2[:], in_=st2[:])
                    sd2 = p3s.tile([128, 1], F32, tag="sd2")
                    nc.scalar.activation(out=sd2[:], in_=mv2[:, 1:2],
                                         func=SQRT, bias=s_eps[:])
                    rstd2 = p3s.tile([128, 1], F32, tag="rstd2")
                    nc.vector.reciprocal(out=rstd2[:], in_=sd2[:])
                    xn2 = p3s.tile([128, 128], F32, tag="xn2")
                    nc.vector.tensor_scalar(out=xn2[:], in0=x2[:],
                                            scalar1=mv2[:, 0:1],
                                            scalar2=rstd2[:],
                                            op0=SUB, op1=MUL)
                    tg = p3s.tile([128, 128], F32, tag="tg")
                    nc.vector.tensor_mul(out=tg[:], in0=xn2[:], in1=s_g2bc[:])
                    ob = p3o.tile([128, 128], F32, tag="ob")
                    nc.vector.tensor_add(out=ob[:], in0=tg[:], in1=s_b2bc[:])
                    nc.sync.dma_start(out=out.ap()[128 * b:128 * (b + 1), :],
                                      in_=ob[:])

                for ch in range(n_ch):
                    g = p2g.tile([128, 3 * cht, 256], F8, tag="g")
                    nc.gpsimd.dma_gather(
                        out_ap=g[:],
                        in_ap=table[:],
                        idxs_ap=s_idx[:, (ch_rows // 16) * ch:
                                      (ch_rows // 16) * (ch + 1)],
                        num_idxs=ch_rows,
                        num_idxs_reg=ch_rows,
                        elem_size=256,
                        single_packet=False,
                        queue_num=ch % 4,
                    )
                    e = p2e.tile([128, ch_rows], BF, tag="e")
                    qs[ch % 2].dma_start(out=e[:],
                                         in_=edge_hT.ap()[:, ch_rows * ch:
                                                          ch_rows * (ch + 1)])
                    for tt in range(cht):
                        t = ch * cht + tt
                        Ds = []
                        for c in range(3):
                            D = p2s.tile([128, 128], BF, tag=f"D{c}")
                            nc.vector.tensor_scalar_mul(
                                out=D[:], in0=s_id[:],
                                scalar1=s_maskc[:, 3 * t + c:3 * t + c + 1])
                            Ds.append(D)
                        ps1 = pp1.tile([128, RT], F32, tag="ps1")
                        nc.tensor.matmul(
                            out=ps1[:],
                            lhsT=s_wc[:],
                            rhs=e[:, RT * tt:RT * (tt + 1)],
                            start=True, stop=False)
                        nb = s_nhTl[:, 8 * t:8 * t + 8]
                        rep = bass.AP(tensor=nb.tensor, offset=nb.offset,
                                      ap=[nb.ap[0], nb.ap[1], [0, KK]])
                        nc.tensor.matmul(out=ps1[:], lhsT=s_wa[:], rhs=rep,
                                         start=False, stop=False)
                        for c in range(3):
                            sub = 3 * tt + c
                            nc.tensor.matmul(out=ps1[:, 128 * c:128 * (c + 1)],
                                             lhsT=g[:, sub, 0:128],
                                             rhs=s_id[:],
                                             start=False, stop=False)
                            nc.tensor.matmul(out=ps1[:, 128 * c:128 * (c + 1)],
                                             lhsT=g[:, sub, 128:256],
                                             rhs=Ds[c][:],
                                             start=False, stop=(c == 2))
                        t2 = p2s.tile([128, RT], BF, tag="t2")
                        nc.scalar.activation(out=t2[:], in_=ps1[:], func=GELU,
                                             bias=s_mb1c[:])
                        ps3 = pp2.tile([128, RT], F32, tag="ps3")
                        nc.tensor.matmul(out=ps3[:], lhsT=s_w2[:], rhs=t2[:],
                                         start=True, stop=True)
                        t4 = p2s.tile([128, RT], BF, tag="t4")
                        nc.scalar.activation(out=t4[:], in_=ps3[:], func=GELU,
                                             bias=s_mb2c[:])
                        with nc.allow_low_precision(
                                reason="48-way bf16 sum feeds LN; 0.4% ok"):
                            nc.vector.reduce_sum(
                                out=s_aggTb[:, 8 * t:8 * (t + 1)],
                                in_=t4[:].rearrange("p (n k) -> p n k", k=KK),
                                axis=AX)
                    # block b's agg completes after chunk (b+1)*ch_per_blk-1;
                    # stage A (mm3+LN1) fires 2 chunks later, A2 (h1
                    # transpose) and B (FF+LN2) on the following chunks so
                    # every PE op has long-ready inputs (no head-of-line)
                    if ch >= ch_per_blk + 1 and ch % ch_per_blk == 1:
                        phase3_blockA((ch - ch_per_blk - 1) // ch_per_blk)
                    if ch >= ch_per_blk + 2 and ch % ch_per_blk == 2:
                        phase3_blockA2((ch - ch_per_blk - 2) // ch_per_blk)
                    if ch >= ch_per_blk + 3 and ch % ch_per_blk == 3:
                        phase3_blockB((ch - ch_per_blk - 3) // ch_per_blk)
                phase3_blockA(nblk - 1)
                phase3_blockA2(nblk - 1)
                phase3_blockB(nblk - 1)

    nc.compile()
    return nc


def prep_core_inputs(inputs, n_glob, n_loc, core):
    """Host-side layout prep for one core. Pure layout/slicing/dtype casts +
    tiny constant broadcasts; no kernel math is done on the host."""
    f32 = np.float32
    n0 = core * n_loc
    E = n_loc * KK
    T = E // RT
    eh = np.ascontiguousarray(
        inputs["edge_h"][n0:n0 + n_loc].reshape(E, H).T).astype(BF16)
    # Rotate the global node axis so this core's local nodes come first;
    # gather indices are rotated to match (table row r = global node
    # (n0 + r) % n_glob).
    j = (inputs["edge_idx"][n0:n0 + n_loc].reshape(E) - n0) % n_glob
    idx16 = np.tile(np.ascontiguousarray(j.reshape(E // 16, 16).T), (8, 1)
                    ).astype(np.int16)
    m = inputs["ar_mask"][n0:n0 + n_loc].reshape(E)
    maskc = np.ascontiguousarray(m.reshape(3 * T, 128).T).astype(f32)
    node_hT = np.ascontiguousarray(
        np.roll(inputs["node_h"], -n0, axis=0).T).astype(BF16)
    seqT = np.ascontiguousarray(
        np.roll(inputs["seq_emb"], -n0, axis=0).T).astype(BF16)
    nhl = np.ascontiguousarray(
        inputs["node_h"][n0:n0 + n_loc].reshape(n_loc // 128, 128, H)
        .transpose(1, 0, 2)).astype(f32)
    mW1 = inputs["mW1"]
    d = {
        "edge_hT": eh, "idx16": idx16, "maskc": maskc,
        "node_hT": node_hT, "seqT": seqT, "nhl": nhl,
        "wa": mW1[0:128].astype(BF16), "wb": mW1[128:256].astype(BF16),
        "wc": mW1[256:384].astype(BF16),
        "wd": mW1[384:512].astype(BF16),
        "w2": inputs["mW2"].astype(BF16), "w3": inputs["mW3"].astype(BF16),
        "fw1": inputs["fW1"].astype(BF16),
        "fw2": np.ascontiguousarray(
            inputs["fW2"].reshape(4, 128, H).transpose(1, 0, 2)).astype(BF16),
        "ident": np.eye(128, dtype=BF16),
        "mb1c": inputs["mb1"].reshape(H, 1).astype(f32),
        "mb2c": inputs["mb2"].reshape(H, 1).astype(f32),
        "mb3x48": (inputs["mb3"] * KK).reshape(H, 1).astype(f32),
        "fb1c": np.ascontiguousarray(
            inputs["fb1"].reshape(4, 128).T).astype(f32),
        "fb2c": inputs["fb2"].reshape(H, 1).astype(f32),
        "g1bc": np.tile(inputs["g1"][None, :], (128, 1)).astype(BF16),
        "b1bc": np.tile(inputs["b1"][None, :], (128, 1)).astype(BF16),
        "g2bc": np.tile(inputs["g2"][None, :], (128, 1)).astype(f32),
        "b2bc": np.tile(inputs["b2"][None, :], (128, 1)).astype(f32),
    }
    return d


_NC_CACHE = {}


def kernel(**inputs):
    inputs = {k: np.asarray(v) for k, v in inputs.items()}
    n_glob = inputs["node_h"].shape[0]
    n_loc = n_glob // N_CORES
    key = (n_glob, n_loc)
    if key not in _NC_CACHE:
        _NC_CACHE[key] = build_nc(n_glob, n_loc, N_CORES)
    nc = _NC_CACHE[key]
    in_maps = [prep_core_inputs(inputs, n_glob, n_loc, c)
               for c in range(N_CORES)]
    res = bass_utils.run_bass_kernel_spmd(nc, in_maps,
                                          core_ids=list(range(N_CORES)))
    return np.concatenate([res.results[c]["out"] for c in range(N_CORES)],
                          axis=0).astype(np.float32)


# revision 31
# speedup vs baseline: 1.0395x; 1.0395x over previous
"""Trainium2 Bass kernel for nn_DecoderLayer (GNN message passing decoder layer).

Math (per reference):
  seq_j = seq_emb[edge_idx] * ar_mask[..., None]
  x = concat([h_i, h_j, edge_h, seq_j], -1)            # [res,k,4h]
  msg = gelu(x @ mW1 + mb1); msg = gelu(msg @ mW2 + mb2); msg = msg @ mW3 + mb3
  agg = msg.sum(1)
  h = LN(node_h + agg) * g1 + b1
  ff = gelu(h @ fW1 + fb1) @ fW2 + fb2
  h = LN(h + ff) * g2 + b2

Strategy (8-way data parallel over the residue dim, no collectives):
  - mm1 is decomposed: x@mW1 = h_i@Wa + h_j@Wb + edge_h@Wc + seq_j@Wd.
    h_j@Wb and seq_emb@Wd are precomputed per *global* node into a fused
    fp8e4m3 gather table [8192, 256]; per-edge rows are fetched with
    dma_gather (alternating SWDGE queues so Q7 descriptor prep overlaps
    the DMA drain) and transpose-accumulated into PSUM via identity
    matmuls; the ar_mask folds into the seq-half transpose as a
    diag(mask) rhs built on the vector engine.
  - k-reduction is moved before mm3 (linearity): 48x less mm3 work.
  - activations live feature-major ("T layout", [feat, rows]); edge_h,
    node_hT and seqT are host-cast to bf16 to halve HBM traffic, and the
    big resident loads are split across engine queues for bandwidth.
  - phase 3 (mm3/LN1/FF/LN2) is interleaved into the edge-chunk loop so
    it overlaps with later chunks' gathers.
"""

import sys

sys.path.insert(0, "/opt/trn_rl_repo")

import numpy as np
import ml_dtypes

import concourse.bacc as bacc
import concourse.bass as bass
import concourse.mybir as mybir
import concourse.tile as tile
from concourse import bass_utils

BF16 = ml_dtypes.bfloat16
F32 = mybir.dt.float32
BF = mybir.dt.bfloat16
I16 = mybir.dt.int16
F8 = mybir.dt.float8e4

RES, KK, H = 8192, 48, 128
N_CORES = 8
RT = 384  # rows per psum tile (8 nodes x 48 edges)


def build_nc(n_glob, n_loc, num_devices, chunk_tiles=4):
    """Build the bass program for one core holding n_loc nodes of an
    n_glob-node graph. All sizes in nodes; n_loc % 128 == 0, n_glob % 128 == 0."""
    E = n_loc * KK
    assert E % RT == 0
    T = E // RT  # number of 384-row tiles
    n_ch = (T + chunk_tiles - 1) // chunk_tiles
    assert T % n_ch == 0
    cht = T // n_ch  # tiles per chunk
    ch_rows = cht * RT  # rows per chunk (gather num_idxs)
    nblk = n_loc // 128  # node blocks
    gblk = n_glob // 128  # global node blocks (table build)
    ch_per_blk = n_ch // nblk  # chunks per 128-node block

    nc = bacc.Bacc("TRN2", target_bir_lowering=False, debug=False,
                   num_devices=num_devices, num_swdge_queues=4)

    def din(name, shape, dt):
        return nc.dram_tensor(name, shape, dt, kind="ExternalInput")

    edge_hT = din("edge_hT", [H, E], BF)
    idx16 = din("idx16", [128, E // 16], I16)
    maskc = din("maskc", [128, 3 * T], F32)
    node_hT = din("node_hT", [H, n_glob], BF)
    seqT = din("seqT", [H, n_glob], BF)
    nhl = din("nhl", [128, nblk, H], F32)
    wa = din("wa", [H, H], BF)
    wb = din("wb", [H, H], BF)
    wc = din("wc", [H, H], BF)
    wd = din("wd", [H, H], BF)
    w2 = din("w2", [H, H], BF)
    w3 = din("w3", [H, H], BF)
    fw1 = din("fw1", [H, 4 * H], BF)
    fw2 = din("fw2", [H, 4, H], BF)
    ident = din("ident", [128, 128], BF)
    mb1c = din("mb1c", [H, 1], F32)
    mb2c = din("mb2c", [H, 1], F32)
    mb3x48 = din("mb3x48", [H, 1], F32)
    fb1c = din("fb1c", [H, 4], F32)
    fb2c = din("fb2c", [H, 1], F32)
    g1bc = din("g1bc", [128, H], BF)
    b1bc = din("b1bc", [128, H], BF)
    g2bc = din("g2bc", [128, H], F32)
    b2bc = din("b2bc", [128, H], F32)
    out = nc.dram_tensor("out", [n_loc, H], F32, kind="ExternalOutput")

    GELU = mybir.ActivationFunctionType.Gelu
    IDENT = mybir.ActivationFunctionType.Identity
    COPY = mybir.ActivationFunctionType.Copy
    SQRT = mybir.ActivationFunctionType.Sqrt
    AX = mybir.AxisListType.X
    SUB = mybir.AluOpType.subtract
    MUL = mybir.AluOpType.mult

    with tile.TileContext(nc) as tc:
        with tc.tile_pool(name="singles", bufs=1) as sg, \
             tc.tile_pool(name="dram", bufs=1, space="DRAM") as dp:
            # ---- resident tiles; big loads are split across engine queues ----
            qs = [nc.sync, nc.scalar]
            s_idx = sg.tile([128, E // 16], I16)
            nq = E // 16 // 4
            for i in range(4):
                qs[i % 2].dma_start(out=s_idx[:, nq * i:nq * (i + 1)],
                                    in_=idx16.ap()[:, nq * i:nq * (i + 1)])
            nhT_bf = sg.tile([128, n_glob], BF)
            seT_bf = sg.tile([128, n_glob], BF)
            gq = n_glob // 16
            for i in range(16):
                qs[i % 2].dma_start(out=nhT_bf[:, gq * i:gq * (i + 1)],
                                    in_=node_hT.ap()[:, gq * i:gq * (i + 1)])
                qs[(i + 1) % 2].dma_start(out=seT_bf[:, gq * i:gq * (i + 1)],
                                          in_=seqT.ap()[:, gq * i:gq * (i + 1)])
            s_wb = sg.tile([H, H], BF)
            nc.sync.dma_start(out=s_wb[:], in_=wb.ap())
            s_wd = sg.tile([H, H], BF)
            nc.sync.dma_start(out=s_wd[:], in_=wd.ap())
            s_id = sg.tile([128, 128], BF)
            nc.sync.dma_start(out=s_id[:], in_=ident.ap())

            def late_loads():
                d = {}
                d["s_maskc"] = sg.tile([128, 3 * T], F32, name="s_maskc")
                nc.sync.dma_start(out=d["s_maskc"][:], in_=maskc.ap())
                d["s_nhl"] = sg.tile([128, nblk, H], F32, name="s_nhl")
                nc.scalar.dma_start(out=d["s_nhl"][:], in_=nhl.ap())
                d["s_wa"] = sg.tile([H, H], BF, name="s_wa")
                nc.sync.dma_start(out=d["s_wa"][:], in_=wa.ap())
                d["s_wc"] = sg.tile([H, H], BF, name="s_wc")
                nc.sync.dma_start(out=d["s_wc"][:], in_=wc.ap())
                d["s_w2"] = sg.tile([H, H], BF, name="s_w2")
                nc.sync.dma_start(out=d["s_w2"][:], in_=w2.ap())
                d["s_w3"] = sg.tile([H, H], BF, name="s_w3")
                nc.sync.dma_start(out=d["s_w3"][:], in_=w3.ap())
                d["s_fw1"] = sg.tile([H, 4 * H], BF, name="s_fw1")
                nc.scalar.dma_start(out=d["s_fw1"][:], in_=fw1.ap())
                d["s_fw2"] = sg.tile([H, 4, H], BF, name="s_fw2")
                nc.scalar.dma_start(out=d["s_fw2"][:], in_=fw2.ap())
                d["s_mb1c"] = sg.tile([H, 1], F32, name="s_mb1c")
                nc.sync.dma_start(out=d["s_mb1c"][:], in_=mb1c.ap())
                d["s_mb2c"] = sg.tile([H, 1], F32, name="s_mb2c")
                nc.sync.dma_start(out=d["s_mb2c"][:], in_=mb2c.ap())
                d["s_mb3x48"] = sg.tile([H, 1], F32, name="s_mb3x48")
                nc.sync.dma_start(out=d["s_mb3x48"][:], in_=mb3x48.ap())
                d["s_fb1c"] = sg.tile([H, 4], F32, name="s_fb1c")
                nc.sync.dma_start(out=d["s_fb1c"][:], in_=fb1c.ap())
                d["s_fb2c"] = sg.tile([H, 1], F32, name="s_fb2c")
                nc.sync.dma_start(out=d["s_fb2c"][:], in_=fb2c.ap())
                d["s_g1bc"] = sg.tile([128, H], BF, name="s_g1bc")
                nc.scalar.dma_start(out=d["s_g1bc"][:], in_=g1bc.ap())
                d["s_b1bc"] = sg.tile([128, H], BF, name="s_b1bc")
                nc.scalar.dma_start(out=d["s_b1bc"][:], in_=b1bc.ap())
                d["s_g2bc"] = sg.tile([128, H], F32, name="s_g2bc")
                nc.scalar.dma_start(out=d["s_g2bc"][:], in_=g2bc.ap())
                d["s_b2bc"] = sg.tile([128, H], F32, name="s_b2bc")
                nc.scalar.dma_start(out=d["s_b2bc"][:], in_=b2bc.ap())
                d["s_eps"] = sg.tile([128, 1], F32, name="s_eps")
                nc.vector.memset(d["s_eps"][:], 1e-5)
                d["s_nhTl"] = sg.tile([128, n_loc], BF, name="s_nhTl")
                nc.vector.tensor_copy(out=d["s_nhTl"][:], in_=nhT_bf[:, 0:n_loc])
                return d
            s_aggT = sg.tile([128, n_loc], F32)
            s_aggTb = sg.tile([128, n_loc], BF)
            s_a2Tb = sg.tile([128, n_loc], BF)
            s_h1T = sg.tile([128, n_loc], BF)
            s_h1rm = sg.tile([128, nblk, H], BF)

            table = dp.tile([n_glob, 256], F8)

            # ---- phase 1: gather table build (deep pipeline) ----
            with tc.tile_pool(name="p1s", bufs=6) as p1s, \
                 tc.tile_pool(name="p1p", bufs=6, space="PSUM") as p1p:
                for b in range(gblk):
                    ps = p1p.tile([128, 256], F32, tag="tps")
                    nc.tensor.matmul(out=ps[:, 0:128],
                                     lhsT=nhT_bf[:, 128 * b:128 * (b + 1)],
                                     rhs=s_wb[:], start=True, stop=True)
                    nc.tensor.matmul(out=ps[:, 128:256],
                                     lhsT=seT_bf[:, 128 * b:128 * (b + 1)],
                                     rhs=s_wd[:], start=True, stop=True)
                    tb = p1s.tile([128, 256], F8, tag="tb")
                    if b % 2 == 0:
                        nc.vector.tensor_copy(out=tb[:], in_=ps[:])
                    else:
                        nc.scalar.activation(out=tb[:], in_=ps[:], func=COPY)
                    (qs + [nc.gpsimd])[b % 3].dma_start(
                        out=table[128 * b:128 * (b + 1), :], in_=tb[:])

            lt = late_loads()
            s_maskc = lt["s_maskc"]; s_nhl = lt["s_nhl"]
            s_wa = lt["s_wa"]; s_wc = lt["s_wc"]; s_w2 = lt["s_w2"]
            s_w3 = lt["s_w3"]; s_fw1 = lt["s_fw1"]; s_fw2 = lt["s_fw2"]
            s_mb1c = lt["s_mb1c"]; s_mb2c = lt["s_mb2c"]
            s_mb3x48 = lt["s_mb3x48"]; s_fb1c = lt["s_fb1c"]
            s_fb2c = lt["s_fb2c"]; s_g1bc = lt["s_g1bc"]
            s_b1bc = lt["s_b1bc"]; s_g2bc = lt["s_g2bc"]
            s_b2bc = lt["s_b2bc"]; s_eps = lt["s_eps"]
            s_nhTl = lt["s_nhTl"]

            # ---- phase 2 + interleaved phase 3 ----
            with tc.tile_pool(name="p2g", bufs=5) as p2g, \
                 tc.tile_pool(name="p2e", bufs=4) as p2e, \
                 tc.tile_pool(name="p2s", bufs=3) as p2s, \
                 tc.tile_pool(name="pp1", bufs=2, space="PSUM") as pp1, \
                 tc.tile_pool(name="pp2", bufs=2, space="PSUM") as pp2, \
                 tc.tile_pool(name="p3s", bufs=2) as p3s, \
                 tc.tile_pool(name="p3o", bufs=2) as p3o, \
                 tc.tile_pool(name="pp3", bufs=2, space="PSUM") as pp3, \
                 tc.tile_pool(name="pp4", bufs=1, space="PSUM") as pp4:

                def newton_rstd(mv, tagp):
                    """rstd = (var+eps)^-1/2 on DVE only (no ACT table swap):
                    reciprocal seed + 5 Newton iterations."""
                    vh = p3s.tile([128, 1], F32, tag=tagp + "vh")
                    nc.vector.tensor_scalar(out=vh[:], in0=mv[:, 1:2],
                                            scalar1=0.5, scalar2=5e-6,
                                            op0=MUL, op1=mybir.AluOpType.add)
                    y = p3s.tile([128, 1], F32, tag=tagp + "y")
                    nc.vector.reciprocal(out=y[:], in_=mv[:, 1:2])
                    for _ in range(5):
                        t = p3s.tile([128, 1], F32, tag=tagp + "t")
                        nc.vector.tensor_mul(out=t[:], in0=y[:], in1=y[:])
                        nc.vector.tensor_mul(out=t[:], in0=t[:], in1=vh[:])
                        nc.vector.tensor_scalar(out=t[:], in0=t[:],
                                                scalar1=-1.0, scalar2=1.5,
                                                op0=MUL,
                                                op1=mybir.AluOpType.add)
                        nc.vector.tensor_mul(out=y[:], in0=y[:], in1=t[:])
                    return y

                def phase3_blockA(b):
                    """mm3 + LN1 for 128-node block b."""
                    psm = pp3.tile([128, 128], F32, tag="p3t")
                    nc.tensor.matmul(out=psm[:], lhsT=s_w3[:],
                                     rhs=s_aggTb[:, 128 * b:128 * (b + 1)],
                                     start=True, stop=True)
                    nc.scalar.activation(out=s_a2Tb[:, 128 * b:128 * (b + 1)],
                                         in_=psm[:], func=IDENT,
                                         bias=s_mb3x48[:])
                    # LN1
                    psrm = pp3.tile([128, 128], F32, tag="p3t")
                    nc.tensor.matmul(out=psrm[:],
                                     lhsT=s_a2Tb[:, 128 * b:128 * (b + 1)],
                                     rhs=s_id[:], start=True, stop=True)
                    x1 = p3s.tile([128, 128], F32, tag="x1")
                    nc.vector.tensor_add(out=x1[:], in0=psrm[:],
                                         in1=s_nhl[:, b, :])
                    st = p3s.tile([128, 6], F32, tag="st")
                    nc.vector.bn_stats(out=st[:], in_=x1[:])
                    mv = p3s.tile([128, 2], F32, tag="mv")
                    nc.vector.bn_aggr(out=mv[:], in_=st[:])
                    sd = p3s.tile([128, 1], F32, tag="sd")
                    nc.scalar.activation(out=sd[:], in_=mv[:, 1:2], func=SQRT,
                                         bias=s_eps[:])
                    rstd = p3s.tile([128, 1], F32, tag="rstd")
                    nc.vector.reciprocal(out=rstd[:], in_=sd[:])
                    xn = p3s.tile([128, 128], BF, tag="xn")
                    nc.vector.tensor_scalar(out=xn[:], in0=x1[:],
                                            scalar1=mv[:, 0:1], scalar2=rstd[:],
                                            op0=SUB, op1=MUL)
                    tb1 = p3s.tile([128, 128], BF, tag="tb1")
                    nc.vector.tensor_mul(out=tb1[:], in0=xn[:], in1=s_g1bc[:])
                    nc.vector.tensor_add(out=s_h1rm[:, b, :], in0=tb1[:],
                                         in1=s_b1bc[:])

                def phase3_blockA2(b):
                    """h1 transpose for block b (inputs one chunk old)."""
                    psT = pp3.tile([128, 128], F32, tag="p3t")
                    nc.tensor.matmul(out=psT[:], lhsT=s_h1rm[:, b, :],
                                     rhs=s_id[:], start=True, stop=True)
                    nc.scalar.activation(out=s_h1T[:, 128 * b:128 * (b + 1)],
                                         in_=psT[:], func=COPY)

                def phase3_blockB(b):
                    """FF + LN2 + out DMA for 128-node block b."""
                    us = []
                    for fc in range(4):
                        psf = pp3.tile([128, 128], F32, tag="p3t")
                        nc.tensor.matmul(out=psf[:],
                                         lhsT=s_fw1[:, 128 * fc:128 * (fc + 1)],
                                         rhs=s_h1T[:, 128 * b:128 * (b + 1)],
                                         start=True, stop=True)
                        u = p3s.tile([128, 128], BF, tag=f"u{fc}")
                        nc.scalar.activation(out=u[:], in_=psf[:], func=GELU,
                                             bias=s_fb1c[:, fc:fc + 1])
                        us.append(u)
                    psf2 = pp4.tile([128, 128], F32, tag="psf2")
                    for fc in range(4):
                        nc.tensor.matmul(out=psf2[:], lhsT=s_fw2[:, fc, :],
                                         rhs=us[fc][:], start=(fc == 0),
                                         stop=(fc == 3))
                    ffT = p3s.tile([128, 128], BF, tag="ffT")
                    nc.scalar.activation(out=ffT[:], in_=psf2[:],
                                         func=IDENT, bias=s_fb2c[:])
                    psr2 = pp3.tile([128, 128], F32, tag="p3t")
                    nc.tensor.matmul(out=psr2[:], lhsT=ffT[:], rhs=s_id[:],
                                     start=True, stop=True)
                    ffrm = p3s.tile([128, 128], BF, tag="ffrm")
                    nc.scalar.activation(out=ffrm[:], in_=psr2[:], func=COPY)
                    x2 = p3s.tile([128, 128], F32, tag="x2")
                    nc.vector.tensor_add(out=x2[:], in0=ffrm[:],
                                         in1=s_h1rm[:, b, :])
                    st2 = p3s.tile([128, 6], F32, tag="st2")
                    nc.vector.bn_stats(out=st2[:], in_=x2[:])
                    mv2 = p3s.tile([128, 2], F32, tag="mv2")
                    nc.vector.bn_aggr(out=mv2[:], in_=st2[:])
                    sd2 = p3s.tile([128, 1], F32, tag="sd2")
                    nc.scalar.activation(out=sd2[:], in_=mv2[:, 1:2],
                                         func=SQRT, bias=s_eps[:])
                    rstd2 = p3s.tile([128, 1], F32, tag="rstd2")
                    nc.vector.reciprocal(out=rstd2[:], in_=sd2[:])
                    xn2 = p3s.tile([128, 128], F32, tag="xn2")
                    nc.vector.tensor_scalar(out=xn2[:], in0=x2[:],
                                            scalar1=mv2[:, 0:1],
                                            scalar2=rstd2[:],
                                            op0=SUB, op1=MUL)
                    tg = p3s.tile([128, 128], F32, tag="tg")
                    nc.vector.tensor_mul(out=tg[:], in0=xn2[:], in1=s_g2bc[:])
                    ob = p3o.tile([128, 128], F32, tag="ob")
                    nc.vector.tensor_add(out=ob[:], in0=tg[:], in1=s_b2bc[:])
                    nc.sync.dma_start(out=out.ap()[128 * b:128 * (b + 1), :],
                                      in_=ob[:])

                for ch in range(n_ch):
                    g = p2g.tile([128, 3 * cht, 256], F8, tag="g")
                    nc.gpsimd.dma_gather(
                        out_ap=g[:],
                        in_ap=table[:],
                        idxs_ap=s_idx[:, (ch_rows // 16) * ch:
                                      (ch_rows // 16) * (ch + 1)],
                        num_idxs=ch_rows,
                        num_idxs_reg=ch_rows,
                        elem_size=256,
                        single_packet=False,
                        queue_num=ch % 4,
                    )
                    e = p2e.tile([128, ch_rows], BF, tag="e")
                    qs[ch % 2].dma_start(out=e[:],
                                         in_=edge_hT.ap()[:, ch_rows * ch:
                                                          ch_rows * (ch + 1)])
                    for tt in range(cht):
                        t = ch * cht + tt
                        Ds = []
                        for c in range(3):
                            D = p2s.tile([128, 128], BF, tag=f"D{c}")
                            nc.vector.tensor_scalar_mul(
                                out=D[:], in0=s_id[:],
                                scalar1=s_maskc[:, 3 * t + c:3 * t + c + 1])
                            Ds.append(D)
                        ps1 = pp1.tile([128, RT], F32, tag="ps1")
                        nc.tensor.matmul(
                            out=ps1[:],
                            lhsT=s_wc[:],
                            rhs=e[:, RT * tt:RT * (tt + 1)],
                            start=True, stop=False)
                        nb = s_nhTl[:, 8 * t:8 * t + 8]
                        rep = bass.AP(tensor=nb.tensor, offset=nb.offset,
                                      ap=[nb.ap[0], nb.ap[1], [0, KK]])
                        nc.tensor.matmul(out=ps1[:], lhsT=s_wa[:], rhs=rep,
                                         start=False, stop=False)
                        for c in range(3):
                            sub = 3 * tt + c
                            nc.tensor.matmul(out=ps1[:, 128 * c:128 * (c + 1)],
                                             lhsT=g[:, sub, 0:128],
                                             rhs=s_id[:],
                                             start=False, stop=False)
                            nc.tensor.matmul(out=ps1[:, 128 * c:128 * (c + 1)],
                                             lhsT=g[:, sub, 128:256],
                                             rhs=Ds[c][:],
                                             start=False, stop=(c == 2))
                        t2 = p2s.tile([128, RT], BF, tag="t2")
                        nc.scalar.activation(out=t2[:], in_=ps1[:], func=GELU,
                                             bias=s_mb1c[:])
                        ps3 = pp2.tile([128, RT], F32, tag="ps3")
                        nc.tensor.matmul(out=ps3[:], lhsT=s_w2[:], rhs=t2[:],
                                         start=True, stop=True)
                        t4 = p2s.tile([128, RT], BF, tag="t4")
                        nc.scalar.activation(out=t4[:], in_=ps3[:], func=GELU,
                                             bias=s_mb2c[:])
                        with nc.allow_low_precision(
                                reason="48-way bf16 sum feeds LN; 0.4% ok"):
                            nc.vector.reduce_sum(
                                out=s_aggTb[:, 8 * t:8 * (t + 1)],
                                in_=t4[:].rearrange("p (n k) -> p n k", k=KK),
                                axis=AX)
                    # block b's agg completes after chunk (b+1)*ch_per_blk-1;
                    # stage A (mm3+LN1) fires 2 chunks later, A2 (h1
                    # transpose) and B (FF+LN2) on the following chunks so
                    # every PE op has long-ready inputs (no head-of-line)
                    if ch >= ch_per_blk + 1 and ch % ch_per_blk == 1:
                        phase3_blockA((ch - ch_per_blk - 1) // ch_per_blk)
                    if ch >= ch_per_blk + 2 and ch % ch_per_blk == 2:
                        phase3_blockA2((ch - ch_per_blk - 2) // ch_per_blk)
                    if ch >= ch_per_blk + 3 and ch % ch_per_blk == 3:
                        phase3_blockB((ch - ch_per_blk - 3) // ch_per_blk)
                phase3_blockA(nblk - 1)
                phase3_blockA2(nblk - 1)
                phase3_blockB(nblk - 1)

    nc.compile()
    return nc


def prep_core_inputs(inputs, n_glob, n_loc, core):
    """Host-side layout prep for one core. Pure layout/slicing/dtype casts +
    tiny constant broadcasts; no kernel math is done on the host."""
    f32 = np.float32
    n0 = core * n_loc
    E = n_loc * KK
    T = E // RT
    eh = np.ascontiguousarray(
        inputs["edge_h"][n0:n0 + n_loc].reshape(E, H).T).astype(BF16)
    # Rotate the global node axis so this core's local nodes come first;
    # gather indices are rotated to match (table row r = global node
    # (n0 + r) % n_glob).
    j = (inputs["edge_idx"][n0:n0 + n_loc].reshape(E) - n0) % n_glob
    idx16 = np.tile(np.ascontiguousarray(j.reshape(E // 16, 16).T), (8, 1)
                    ).astype(np.int16)
    m = inputs["ar_mask"][n0:n0 + n_loc].reshape(E)
    maskc = np.ascontiguousarray(m.reshape(3 * T, 128).T).astype(f32)
    node_hT = np.ascontiguousarray(
        np.roll(inputs["node_h"], -n0, axis=0).T).astype(BF16)
    seqT = np.ascontiguousarray(
        np.roll(inputs["seq_emb"], -n0, axis=0).T).astype(BF16)
    nhl = np.ascontiguousarray(
        inputs["node_h"][n0:n0 + n_loc].reshape(n_loc // 128, 128, H)
        .transpose(1, 0, 2)).astype(f32)
    mW1 = inputs["mW1"]
    d = {
        "edge_hT": eh, "idx16": idx16, "maskc": maskc,
        "node_hT": node_hT, "seqT": seqT, "nhl": nhl,
        "wa": mW1[0:128].astype(BF16), "wb": mW1[128:256].astype(BF16),
        "wc": mW1[256:384].astype(BF16),
        "wd": mW1[384:512].astype(BF16),
        "w2": inputs["mW2"].astype(BF16), "w3": inputs["mW3"].astype(BF16),
        "fw1": inputs["fW1"].astype(BF16),
        "fw2": np.ascontiguousarray(
            inputs["fW2"].reshape(4, 128, H).transpose(1, 0, 2)).astype(BF16),
        "ident": np.eye(128, dtype=BF16),
        "mb1c": inputs["mb1"].reshape(H, 1).astype(f32),
        "mb2c": inputs["mb2"].reshape(H, 1).astype(f32),
        "mb3x48": (inputs["mb3"] * KK).reshape(H, 1).astype(f32),
        "fb1c": np.ascontiguousarray(
            inputs["fb1"].reshape(4, 128).T).astype(f32),
        "fb2c": inputs["fb2"].reshape(H, 1).astype(f32),
        "g1bc": np.tile(inputs["g1"][None, :], (128, 1)).astype(BF16),
        "b1bc": np.tile(inputs["b1"][None, :], (128, 1)).astype(BF16),
        "g2bc": np.tile(inputs["g2"][None, :], (128, 1)).astype(f32),
        "b2bc": np.tile(inputs["b2"][None, :], (128, 1)).astype(f32),
    }
    return d


_NC_CACHE = {}


def kernel(**inputs):
    inputs = {k: np.asarray(v) for k, v in inputs.items()}
    n_glob = inputs["node_h"].shape[0]
    n_loc = n_glob // N_CORES
    key = (n_glob, n_loc)
    if key not in _NC_CACHE:
        _NC_CACHE[key] = build_nc(n_glob, n_loc, N_CORES)
    nc = _NC_CACHE[key]
    in_maps = [prep_core_inputs(inputs, n_glob, n_loc, c)
               for c in range(N_CORES)]
    res = bass_utils.run_bass_kernel_spmd(nc, in_maps,
                                          core_ids=list(range(N_CORES)))
    return np.concatenate([res.results[c]["out"] for c in range(N_CORES)],
                          axis=0).astype(np.float32)


# revision 32
# speedup vs baseline: 1.0517x; 1.0117x over previous
"""Trainium2 Bass kernel for nn_DecoderLayer (GNN message passing decoder layer).

Math (per reference):
  seq_j = seq_emb[edge_idx] * ar_mask[..., None]
  x = concat([h_i, h_j, edge_h, seq_j], -1)            # [res,k,4h]
  msg = gelu(x @ mW1 + mb1); msg = gelu(msg @ mW2 + mb2); msg = msg @ mW3 + mb3
  agg = msg.sum(1)
  h = LN(node_h + agg) * g1 + b1
  ff = gelu(h @ fW1 + fb1) @ fW2 + fb2
  h = LN(h + ff) * g2 + b2

Strategy (8-way data parallel over the residue dim, no collectives):
  - mm1 is decomposed: x@mW1 = h_i@Wa + h_j@Wb + edge_h@Wc + seq_j@Wd.
    h_j@Wb and seq_emb@Wd are precomputed per *global* node into a fused
    fp8e4m3 gather table [8192, 256]; per-edge rows are fetched with
    dma_gather (alternating SWDGE queues so Q7 descriptor prep overlaps
    the DMA drain) and transpose-accumulated into PSUM via identity
    matmuls; the ar_mask folds into the seq-half transpose as a
    diag(mask) rhs built on the vector engine.
  - k-reduction is moved before mm3 (linearity): 48x less mm3 work.
  - activations live feature-major ("T layout", [feat, rows]); edge_h,
    node_hT and seqT are host-cast to bf16 to halve HBM traffic, and the
    big resident loads are split across engine queues for bandwidth.
  - phase 3 (mm3/LN1/FF/LN2) is interleaved into the edge-chunk loop so
    it overlaps with later chunks' gathers.
"""

import sys

sys.path.insert(0, "/opt/trn_rl_repo")

import numpy as np
import ml_dtypes

import concourse.bacc as bacc
import concourse.bass as bass
import concourse.mybir as mybir
import concourse.tile as tile
from concourse import bass_utils

BF16 = ml_dtypes.bfloat16
F32 = mybir.dt.float32
BF = mybir.dt.bfloat16
I16 = mybir.dt.int16
F8 = mybir.dt.float8e4

RES, KK, H = 8192, 48, 128
N_CORES = 8
RT = 384  # rows per psum tile (8 nodes x 48 edges)


def build_nc(n_glob, n_loc, num_devices, chunk_tiles=4):
    """Build the bass program for one core holding n_loc nodes of an
    n_glob-node graph. All sizes in nodes; n_loc % 128 == 0, n_glob % 128 == 0."""
    E = n_loc * KK
    assert E % RT == 0
    T = E // RT  # number of 384-row tiles
    n_ch = (T + chunk_tiles - 1) // chunk_tiles
    assert T % n_ch == 0
    cht = T // n_ch  # tiles per chunk
    ch_rows = cht * RT  # rows per chunk (gather num_idxs)
    nblk = n_loc // 128  # node blocks
    gblk = n_glob // 128  # global node blocks (table build)
    ch_per_blk = n_ch // nblk  # chunks per 128-node block

    nc = bacc.Bacc("TRN2", target_bir_lowering=False, debug=False,
                   num_devices=num_devices, num_swdge_queues=4)

    def din(name, shape, dt):
        return nc.dram_tensor(name, shape, dt, kind="ExternalInput")

    edge_hT = din("edge_hT", [H, E], BF)
    idx16 = din("idx16", [128, E // 16], I16)
    maskc = din("maskc", [128, 3 * T], F32)
    node_hT = din("node_hT", [H, n_glob], BF)
    seqT = din("seqT", [H, n_glob], BF)
    nhl = din("nhl", [128, nblk, H], F32)
    wa = din("wa", [H, H], BF)
    wb = din("wb", [H, H], BF)
    wc = din("wc", [H, H], BF)
    wd = din("wd", [H, H], BF)
    w2 = din("w2", [H, H], BF)
    w3 = din("w3", [H, H], BF)
    fw1 = din("fw1", [H, 4 * H], BF)
    fw2 = din("fw2", [H, 4, H], BF)
    ident = din("ident", [128, 128], BF)
    mb1c = din("mb1c", [H, 1], F32)
    mb2c = din("mb2c", [H, 1], F32)
    mb3x48 = din("mb3x48", [H, 1], F32)
    fb1c = din("fb1c", [H, 4], F32)
    fb2c = din("fb2c", [H, 1], F32)
    g1bc = din("g1bc", [128, H], BF)
    b1bc = din("b1bc", [128, H], BF)
    g2bc = din("g2bc", [128, H], F32)
    b2bc = din("b2bc", [128, H], F32)
    out = nc.dram_tensor("out", [n_loc, H], F32, kind="ExternalOutput")

    GELU = mybir.ActivationFunctionType.Gelu
    IDENT = mybir.ActivationFunctionType.Identity
    COPY = mybir.ActivationFunctionType.Copy
    SQRT = mybir.ActivationFunctionType.Sqrt
    AX = mybir.AxisListType.X
    SUB = mybir.AluOpType.subtract
    MUL = mybir.AluOpType.mult

    with tile.TileContext(nc) as tc:
        with tc.tile_pool(name="singles", bufs=1) as sg, \
             tc.tile_pool(name="dram", bufs=1, space="DRAM") as dp:
            # ---- resident tiles; big loads are split across engine queues ----
            qs = [nc.sync, nc.scalar]
            s_idx = sg.tile([128, E // 16], I16)
            nq = E // 16 // 4
            for i in range(4):
                qs[i % 2].dma_start(out=s_idx[:, nq * i:nq * (i + 1)],
                                    in_=idx16.ap()[:, nq * i:nq * (i + 1)])
            nhT_bf = sg.tile([128, n_glob], BF)
            seT_bf = sg.tile([128, n_glob], BF)
            gq = n_glob // 16
            for i in range(16):
                qs[i % 2].dma_start(out=nhT_bf[:, gq * i:gq * (i + 1)],
                                    in_=node_hT.ap()[:, gq * i:gq * (i + 1)])
                qs[(i + 1) % 2].dma_start(out=seT_bf[:, gq * i:gq * (i + 1)],
                                          in_=seqT.ap()[:, gq * i:gq * (i + 1)])
            s_wb = sg.tile([H, H], BF)
            nc.sync.dma_start(out=s_wb[:], in_=wb.ap())
            s_wd = sg.tile([H, H], BF)
            nc.sync.dma_start(out=s_wd[:], in_=wd.ap())
            s_id = sg.tile([128, 128], BF)
            nc.sync.dma_start(out=s_id[:], in_=ident.ap())

            def late_loads():
                d = {}
                d["s_maskc"] = sg.tile([128, 3 * T], F32, name="s_maskc")
                nc.sync.dma_start(out=d["s_maskc"][:], in_=maskc.ap())
                d["s_nhl"] = sg.tile([128, nblk, H], F32, name="s_nhl")
                nc.scalar.dma_start(out=d["s_nhl"][:], in_=nhl.ap())
                d["s_wa"] = sg.tile([H, H], BF, name="s_wa")
                nc.sync.dma_start(out=d["s_wa"][:], in_=wa.ap())
                d["s_wc"] = sg.tile([H, H], BF, name="s_wc")
                nc.sync.dma_start(out=d["s_wc"][:], in_=wc.ap())
                d["s_w2"] = sg.tile([H, H], BF, name="s_w2")
                nc.sync.dma_start(out=d["s_w2"][:], in_=w2.ap())
                d["s_w3"] = sg.tile([H, H], BF, name="s_w3")
                nc.sync.dma_start(out=d["s_w3"][:], in_=w3.ap())
                d["s_fw1"] = sg.tile([H, 4 * H], BF, name="s_fw1")
                nc.scalar.dma_start(out=d["s_fw1"][:], in_=fw1.ap())
                d["s_fw2"] = sg.tile([H, 4, H], BF, name="s_fw2")
                nc.scalar.dma_start(out=d["s_fw2"][:], in_=fw2.ap())
                d["s_mb1c"] = sg.tile([H, 1], F32, name="s_mb1c")
                nc.sync.dma_start(out=d["s_mb1c"][:], in_=mb1c.ap())
                d["s_mb2c"] = sg.tile([H, 1], F32, name="s_mb2c")
                nc.sync.dma_start(out=d["s_mb2c"][:], in_=mb2c.ap())
                d["s_mb3x48"] = sg.tile([H, 1], F32, name="s_mb3x48")
                nc.sync.dma_start(out=d["s_mb3x48"][:], in_=mb3x48.ap())
                d["s_fb1c"] = sg.tile([H, 4], F32, name="s_fb1c")
                nc.sync.dma_start(out=d["s_fb1c"][:], in_=fb1c.ap())
                d["s_fb2c"] = sg.tile([H, 1], F32, name="s_fb2c")
                nc.sync.dma_start(out=d["s_fb2c"][:], in_=fb2c.ap())
                d["s_g1bc"] = sg.tile([128, H], BF, name="s_g1bc")
                nc.scalar.dma_start(out=d["s_g1bc"][:], in_=g1bc.ap())
                d["s_b1bc"] = sg.tile([128, H], BF, name="s_b1bc")
                nc.scalar.dma_start(out=d["s_b1bc"][:], in_=b1bc.ap())
                d["s_g2bc"] = sg.tile([128, H], F32, name="s_g2bc")
                nc.scalar.dma_start(out=d["s_g2bc"][:], in_=g2bc.ap())
                d["s_b2bc"] = sg.tile([128, H], F32, name="s_b2bc")
                nc.scalar.dma_start(out=d["s_b2bc"][:], in_=b2bc.ap())
                d["s_eps"] = sg.tile([128, 1], F32, name="s_eps")
                nc.vector.memset(d["s_eps"][:], 1e-5)
                d["s_nhTl"] = sg.tile([128, n_loc], BF, name="s_nhTl")
                nc.vector.tensor_copy(out=d["s_nhTl"][:], in_=nhT_bf[:, 0:n_loc])
                return d
            s_aggT = sg.tile([128, n_loc], F32)
            s_aggTb = sg.tile([128, n_loc], BF)
            s_a2Tb = sg.tile([128, n_loc], BF)
            s_h1T = sg.tile([128, n_loc], BF)
            s_h1rm = sg.tile([128, nblk, H], BF)

            table = dp.tile([n_glob, 256], F8)

            # ---- phase 1: gather table build (deep pipeline) ----
            with tc.tile_pool(name="p1s", bufs=6) as p1s, \
                 tc.tile_pool(name="p1p", bufs=6, space="PSUM") as p1p:
                for b in range(gblk):
                    ps = p1p.tile([128, 256], F32, tag="tps")
                    nc.tensor.matmul(out=ps[:, 0:128],
                                     lhsT=nhT_bf[:, 128 * b:128 * (b + 1)],
                                     rhs=s_wb[:], start=True, stop=True)
                    nc.tensor.matmul(out=ps[:, 128:256],
                                     lhsT=seT_bf[:, 128 * b:128 * (b + 1)],
                                     rhs=s_wd[:], start=True, stop=True)
                    tb = p1s.tile([128, 256], F8, tag="tb")
                    if b % 2 == 0:
                        nc.vector.tensor_copy(out=tb[:], in_=ps[:])
                    else:
                        nc.scalar.activation(out=tb[:], in_=ps[:], func=COPY)
                    (qs + [nc.gpsimd])[b % 3].dma_start(
                        out=table[128 * b:128 * (b + 1), :], in_=tb[:])

            lt = late_loads()
            s_maskc = lt["s_maskc"]; s_nhl = lt["s_nhl"]
            s_wa = lt["s_wa"]; s_wc = lt["s_wc"]; s_w2 = lt["s_w2"]
            s_w3 = lt["s_w3"]; s_fw1 = lt["s_fw1"]; s_fw2 = lt["s_fw2"]
            s_mb1c = lt["s_mb1c"]; s_mb2c = lt["s_mb2c"]
            s_mb3x48 = lt["s_mb3x48"]; s_fb1c = lt["s_fb1c"]
            s_fb2c = lt["s_fb2c"]; s_g1bc = lt["s_g1bc"]
            s_b1bc = lt["s_b1bc"]; s_g2bc = lt["s_g2bc"]
            s_b2bc = lt["s_b2bc"]; s_eps = lt["s_eps"]
            s_nhTl = lt["s_nhTl"]

            # ---- phase 2 + interleaved phase 3 ----
            with tc.tile_pool(name="p2g", bufs=5) as p2g, \
                 tc.tile_pool(name="p2e", bufs=4) as p2e, \
                 tc.tile_pool(name="p2s", bufs=4) as p2s, \
                 tc.tile_pool(name="pp1", bufs=2, space="PSUM") as pp1, \
                 tc.tile_pool(name="pp2", bufs=2, space="PSUM") as pp2, \
                 tc.tile_pool(name="p3s", bufs=2) as p3s, \
                 tc.tile_pool(name="p3o", bufs=2) as p3o, \
                 tc.tile_pool(name="pp3", bufs=3, space="PSUM") as pp3, \
                 tc.tile_pool(name="pp4", bufs=1, space="PSUM") as pp4:

                def newton_rstd(mv, tagp):
                    """rstd = (var+eps)^-1/2 on DVE only (no ACT table swap):
                    reciprocal seed + 5 Newton iterations."""
                    vh = p3s.tile([128, 1], F32, tag=tagp + "vh")
                    nc.vector.tensor_scalar(out=vh[:], in0=mv[:, 1:2],
                                            scalar1=0.5, scalar2=5e-6,
                                            op0=MUL, op1=mybir.AluOpType.add)
                    y = p3s.tile([128, 1], F32, tag=tagp + "y")
                    nc.vector.reciprocal(out=y[:], in_=mv[:, 1:2])
                    for _ in range(5):
                        t = p3s.tile([128, 1], F32, tag=tagp + "t")
                        nc.vector.tensor_mul(out=t[:], in0=y[:], in1=y[:])
                        nc.vector.tensor_mul(out=t[:], in0=t[:], in1=vh[:])
                        nc.vector.tensor_scalar(out=t[:], in0=t[:],
                                                scalar1=-1.0, scalar2=1.5,
                                                op0=MUL,
                                                op1=mybir.AluOpType.add)
                        nc.vector.tensor_mul(out=y[:], in0=y[:], in1=t[:])
                    return y

                def phase3_blockA(b):
                    """mm3 + LN1 for 128-node block b."""
                    psm = pp3.tile([128, 128], F32, tag="p3t")
                    nc.tensor.matmul(out=psm[:], lhsT=s_w3[:],
                                     rhs=s_aggTb[:, 128 * b:128 * (b + 1)],
                                     start=True, stop=True)
                    nc.scalar.activation(out=s_a2Tb[:, 128 * b:128 * (b + 1)],
                                         in_=psm[:], func=IDENT,
                                         bias=s_mb3x48[:])
                    # LN1
                    psrm = pp3.tile([128, 128], F32, tag="p3t")
                    nc.tensor.matmul(out=psrm[:],
                                     lhsT=s_a2Tb[:, 128 * b:128 * (b + 1)],
                                     rhs=s_id[:], start=True, stop=True)
                    x1 = p3s.tile([128, 128], F32, tag="x1")
                    nc.vector.tensor_add(out=x1[:], in0=psrm[:],
                                         in1=s_nhl[:, b, :])
                    st = p3s.tile([128, 6], F32, tag="st")
                    nc.vector.bn_stats(out=st[:], in_=x1[:])
                    mv = p3s.tile([128, 2], F32, tag="mv")
                    nc.vector.bn_aggr(out=mv[:], in_=st[:])
                    sd = p3s.tile([128, 1], F32, tag="sd")
                    nc.scalar.activation(out=sd[:], in_=mv[:, 1:2], func=SQRT,
                                         bias=s_eps[:])
                    rstd = p3s.tile([128, 1], F32, tag="rstd")
                    nc.vector.reciprocal(out=rstd[:], in_=sd[:])
                    xn = p3s.tile([128, 128], BF, tag="xn")
                    nc.vector.tensor_scalar(out=xn[:], in0=x1[:],
                                            scalar1=mv[:, 0:1], scalar2=rstd[:],
                                            op0=SUB, op1=MUL)
                    tb1 = p3s.tile([128, 128], BF, tag="tb1")
                    nc.vector.tensor_mul(out=tb1[:], in0=xn[:], in1=s_g1bc[:])
                    nc.vector.tensor_add(out=s_h1rm[:, b, :], in0=tb1[:],
                                         in1=s_b1bc[:])

                def phase3_blockA2(b):
                    """h1 transpose for block b (inputs one chunk old)."""
                    psT = pp3.tile([128, 128], F32, tag="p3t")
                    nc.tensor.matmul(out=psT[:], lhsT=s_h1rm[:, b, :],
                                     rhs=s_id[:], start=True, stop=True)
                    nc.scalar.activation(out=s_h1T[:, 128 * b:128 * (b + 1)],
                                         in_=psT[:], func=COPY)

                def phase3_blockB(b):
                    """FF + LN2 + out DMA for 128-node block b."""
                    us = []
                    for fc in range(4):
                        psf = pp3.tile([128, 128], F32, tag="p3t")
                        nc.tensor.matmul(out=psf[:],
                                         lhsT=s_fw1[:, 128 * fc:128 * (fc + 1)],
                                         rhs=s_h1T[:, 128 * b:128 * (b + 1)],
                                         start=True, stop=True)
                        u = p3s.tile([128, 128], BF, tag=f"u{fc}")
                        nc.scalar.activation(out=u[:], in_=psf[:], func=GELU,
                                             bias=s_fb1c[:, fc:fc + 1])
                        us.append(u)
                    psf2 = pp4.tile([128, 128], F32, tag="psf2")
                    for fc in range(4):
                        nc.tensor.matmul(out=psf2[:], lhsT=s_fw2[:, fc, :],
                                         rhs=us[fc][:], start=(fc == 0),
                                         stop=(fc == 3))
                    ffT = p3s.tile([128, 128], BF, tag="ffT")
                    nc.scalar.activation(out=ffT[:], in_=psf2[:],
                                         func=IDENT, bias=s_fb2c[:])
                    psr2 = pp3.tile([128, 128], F32, tag="p3t")
                    nc.tensor.matmul(out=psr2[:], lhsT=ffT[:], rhs=s_id[:],
                                     start=True, stop=True)
                    ffrm = p3s.tile([128, 128], BF, tag="ffrm")
                    nc.scalar.activation(out=ffrm[:], in_=psr2[:], func=COPY)
                    x2 = p3s.tile([128, 128], F32, tag="x2")
                    nc.vector.tensor_add(out=x2[:], in0=ffrm[:],
                                         in1=s_h1rm[:, b, :])
                    st2 = p3s.tile([128, 6], F32, tag="st2")
                    nc.vector.bn_stats(out=st2[:], in_=x2[:])
                    mv2 = p3s.tile([128, 2], F32, tag="mv2")
                    nc.vector.bn_aggr(out=mv2[:], in_=st2[:])
                    sd2 = p3s.tile([128, 1], F32, tag="sd2")
                    nc.scalar.activation(out=sd2[:], in_=mv2[:, 1:2],
                                         func=SQRT, bias=s_eps[:])
                    rstd2 = p3s.tile([128, 1], F32, tag="rstd2")
                    nc.vector.reciprocal(out=rstd2[:], in_=sd2[:])
                    xn2 = p3s.tile([128, 128], F32, tag="xn2")
                    nc.vector.tensor_scalar(out=xn2[:], in0=x2[:],
                                            scalar1=mv2[:, 0:1],
                                            scalar2=rstd2[:],
                                            op0=SUB, op1=MUL)
                    tg = p3s.tile([128, 128], F32, tag="tg")
                    nc.vector.tensor_mul(out=tg[:], in0=xn2[:], in1=s_g2bc[:])
                    ob = p3o.tile([128, 128], F32, tag="ob")
                    nc.vector.tensor_add(out=ob[:], in0=tg[:], in1=s_b2bc[:])
                    nc.sync.dma_start(out=out.ap()[128 * b:128 * (b + 1), :],
                                      in_=ob[:])

                for ch in range(n_ch):
                    g = p2g.tile([128, 3 * cht, 256], F8, tag="g")
                    nc.gpsimd.dma_gather(
                        out_ap=g[:],
                        in_ap=table[:],
                        idxs_ap=s_idx[:, (ch_rows // 16) * ch:
                                      (ch_rows // 16) * (ch + 1)],
                        num_idxs=ch_rows,
                        num_idxs_reg=ch_rows,
                        elem_size=256,
                        single_packet=False,
                        queue_num=ch % 4,
                    )
                    e = p2e.tile([128, ch_rows], BF, tag="e")
                    qs[ch % 2].dma_start(out=e[:],
                                         in_=edge_hT.ap()[:, ch_rows * ch:
                                                          ch_rows * (ch + 1)])
                    for tt in range(cht):
                        t = ch * cht + tt
                        Ds = []
                        for c in range(3):
                            D = p2s.tile([128, 128], BF, tag=f"D{c}")
                            nc.vector.tensor_scalar_mul(
                                out=D[:], in0=s_id[:],
                                scalar1=s_maskc[:, 3 * t + c:3 * t + c + 1])
                            Ds.append(D)
                        ps1 = pp1.tile([128, RT], F32, tag="ps1")
                        nc.tensor.matmul(
                            out=ps1[:],
                            lhsT=s_wc[:],
                            rhs=e[:, RT * tt:RT * (tt + 1)],
                            start=True, stop=False)
                        nb = s_nhTl[:, 8 * t:8 * t + 8]
                        rep = bass.AP(tensor=nb.tensor, offset=nb.offset,
                                      ap=[nb.ap[0], nb.ap[1], [0, KK]])
                        nc.tensor.matmul(out=ps1[:], lhsT=s_wa[:], rhs=rep,
                                         start=False, stop=False)
                        for c in range(3):
                            sub = 3 * tt + c
                            nc.tensor.matmul(out=ps1[:, 128 * c:128 * (c + 1)],
                                             lhsT=g[:, sub, 0:128],
                                             rhs=s_id[:],
                                             start=False, stop=False)
                            nc.tensor.matmul(out=ps1[:, 128 * c:128 * (c + 1)],
                                             lhsT=g[:, sub, 128:256],
                                             rhs=Ds[c][:],
                                             start=False, stop=(c == 2))
                        t2 = p2s.tile([128, RT], BF, tag="t2")
                        nc.scalar.activation(out=t2[:], in_=ps1[:], func=GELU,
                                             bias=s_mb1c[:])
                        ps3 = pp2.tile([128, RT], F32, tag="ps3")
                        nc.tensor.matmul(out=ps3[:], lhsT=s_w2[:], rhs=t2[:],
                                         start=True, stop=True)
                        t4 = p2s.tile([128, RT], BF, tag="t4")
                        nc.scalar.activation(out=t4[:], in_=ps3[:], func=GELU,
                                             bias=s_mb2c[:])
                        with nc.allow_low_precision(
                                reason="48-way bf16 sum feeds LN; 0.4% ok"):
                            nc.vector.reduce_sum(
                                out=s_aggTb[:, 8 * t:8 * (t + 1)],
                                in_=t4[:].rearrange("p (n k) -> p n k", k=KK),
                                axis=AX)
                    # block b's agg completes after chunk (b+1)*ch_per_blk-1;
                    # stage A (mm3+LN1) fires 2 chunks later, A2 (h1
                    # transpose) and B (FF+LN2) on the following chunks so
                    # every PE op has long-ready inputs (no head-of-line)
                    if ch >= ch_per_blk + 1 and ch % ch_per_blk == 1:
                        phase3_blockA((ch - ch_per_blk - 1) // ch_per_blk)
                    if ch >= ch_per_blk + 2 and ch % ch_per_blk == 2:
                        phase3_blockA2((ch - ch_per_blk - 2) // ch_per_blk)
                    if ch >= ch_per_blk + 3 and ch % ch_per_blk == 3:
                        phase3_blockB((ch - ch_per_blk - 3) // ch_per_blk)
                phase3_blockA(nblk - 1)
                phase3_blockA2(nblk - 1)
                phase3_blockB(nblk - 1)

    nc.compile()
    return nc


def prep_core_inputs(inputs, n_glob, n_loc, core):
    """Host-side layout prep for one core. Pure layout/slicing/dtype casts +
    tiny constant broadcasts; no kernel math is done on the host."""
    f32 = np.float32
    n0 = core * n_loc
    E = n_loc * KK
    T = E // RT
    eh = np.ascontiguousarray(
        inputs["edge_h"][n0:n0 + n_loc].reshape(E, H).T).astype(BF16)
    # Rotate the global node axis so this core's local nodes come first;
    # gather indices are rotated to match (table row r = global node
    # (n0 + r) % n_glob).
    j = (inputs["edge_idx"][n0:n0 + n_loc].reshape(E) - n0) % n_glob
    idx16 = np.tile(np.ascontiguousarray(j.reshape(E // 16, 16).T), (8, 1)
                    ).astype(np.int16)
    m = inputs["ar_mask"][n0:n0 + n_loc].reshape(E)
    maskc = np.ascontiguousarray(m.reshape(3 * T, 128).T).astype(f32)
    node_hT = np.ascontiguousarray(
        np.roll(inputs["node_h"], -n0, axis=0).T).astype(BF16)
    seqT = np.ascontiguousarray(
        np.roll(inputs["seq_emb"], -n0, axis=0).T).astype(BF16)
    nhl = np.ascontiguousarray(
        inputs["node_h"][n0:n0 + n_loc].reshape(n_loc // 128, 128, H)
        .transpose(1, 0, 2)).astype(f32)
    mW1 = inputs["mW1"]
    d = {
        "edge_hT": eh, "idx16": idx16, "maskc": maskc,
        "node_hT": node_hT, "seqT": seqT, "nhl": nhl,
        "wa": mW1[0:128].astype(BF16), "wb": mW1[128:256].astype(BF16),
        "wc": mW1[256:384].astype(BF16),
        "wd": mW1[384:512].astype(BF16),
        "w2": inputs["mW2"].astype(BF16), "w3": inputs["mW3"].astype(BF16),
        "fw1": inputs["fW1"].astype(BF16),
        "fw2": np.ascontiguousarray(
            inputs["fW2"].reshape(4, 128, H).transpose(1, 0, 2)).astype(BF16),
        "ident": np.eye(128, dtype=BF16),
        "mb1c": inputs["mb1"].reshape(H, 1).astype(f32),
        "mb2c": inputs["mb2"].reshape(H, 1).astype(f32),
        "mb3x48": (inputs["mb3"] * KK).reshape(H, 1).astype(f32),
        "fb1c": np.ascontiguousarray(
            inputs["fb1"].reshape(4, 128).T).astype(f32),
        "fb2c": inputs["fb2"].reshape(H, 1).astype(f32),
        "g1bc": np.tile(inputs["g1"][None, :], (128, 1)).astype(BF16),
        "b1bc": np.tile(inputs["b1"][None, :], (128, 1)).astype(BF16),
        "g2bc": np.tile(inputs["g2"][None, :], (128, 1)).astype(f32),
        "b2bc": np.tile(inputs["b2"][None, :], (128, 1)).astype(f32),
    }
    return d


_NC_CACHE = {}


def kernel(**inputs):
    inputs = {k: np.asarray(v) for k, v in inputs.items()}
    n_glob = inputs["node_h"].shape[0]
    n_loc = n_glob // N_CORES
    key = (n_glob, n_loc)
    if key not in _NC_CACHE:
        _NC_CACHE[key] = build_nc(n_glob, n_loc, N_CORES)
    nc = _NC_CACHE[key]
    in_maps = [prep_core_inputs(inputs, n_glob, n_loc, c)
               for c in range(N_CORES)]
    res = bass_utils.run_bass_kernel_spmd(nc, in_maps,
                                          core_ids=list(range(N_CORES)))
    return np.concatenate([res.results[c]["out"] for c in range(N_CORES)],
                          axis=0).astype(np.float32)
